# revision 8
# baseline (speedup 1.0000x reference)
"""Multi-head causal attention (B=2, T=2048, D=1024, H=16, dk=dv=64) on 8 NeuronCores.

Sharding: data parallel over batch (2) x tensor parallel over heads (4 groups of 4).
Core c handles batch c//4, heads [4*(c%4), 4*(c%4)+4). Each core computes the
partial output sum over its 4 heads; host adds the 4 partials per batch.

Per-core pipeline (everything transposed so no activation transposes are needed):
  QT/KT [256, T] = W.T @ xT        (fp32r matmuls, PSUM k-accumulation)
  VT    [256, T] -> PE-transpose -> V_aug [T, 65] per head (65th col = ones)
  per head, per tq-slice (512), per tk-tile (128), causal-skipped:
    ST block [tk 128, tq 512] = KT_h^T-slice.T @ QT_h      (K=64)
    diag blocks: += additive -1e30 mask (DVE on PSUM)
    ET = exp(0.125 * ST)  (ACT, PSUM->SBUF fp32r, batched over 2 blocks)
    OT_aug [65, 512] += V_aug_h.T-slice @ ET   (fused rowsum via ones col)
  normalization: rowsums -> DRAM bounce broadcast -> reciprocal -> OT scale
  out [T, 1024] = OT.T @ Wo  (partial over this core's 4 heads)
"""
import sys

sys.path.insert(0, "/opt/trn_rl_repo")

import functools
import numpy as np

import concourse.bass as bass
import concourse.tile as tile
from concourse import mybir
from concourse.masks import make_identity
from concourse.bass_utils import run_bass_kernel_spmd

B, T, D = 2, 2048, 1024
H, DK = 16, 64            # total heads
HG = 4                    # heads per core
G = HG * DK               # 256: per-core column group width
NKT = D // 128            # 8 k-tiles of the model dim
NT = T // 128             # 16 tk tiles
NS = 4                    # tq slices
TQ = T // NS              # 512
NEG = -1e30
F32 = mybir.dt.float32
F32R = mybir.dt.float32r


def split_multi_waits(nc, max_waits=1):
    """This walrus build has tiny per-instruction sync-wait slot limits (1 for
    fp32r matmul LW, ~2 for CTRL). Move excess waits onto preceding same-engine
    NOPs - identical semantics since each engine executes serially."""
    for func in nc.m.functions:
        for bb in func.blocks:
            out = []
            for inst in list(bb.instructions):
                si = inst.sync_info
                waits = list(si.on_wait) if (si and si.on_wait) else []
                if len(waits) > max_waits:
                    extra, keep = waits[:-max_waits], waits[-max_waits:]
                    for j, w in enumerate(extra):
                        nop = mybir.InstNoOp(name=f"{inst.name}-ws{j}")
                        nop.engine = inst.engine
                        nop.sync_info = mybir.SyncInfo(on_wait=[w], on_update=[])
                        out.append(nop)
                    inst.sync_info = mybir.SyncInfo(
                        on_wait=keep, on_update=list(si.on_update or []))
                out.append(inst)
            bb.instructions = out


def _n_alive(s, mode):
    """Number of tk tiles needed for tq slice s."""
    return NT if mode != "causal" else (TQ // 128) * (s + 1)


@functools.lru_cache(maxsize=4)
def build_program(mode):
    assert mode in ("causal", "dense", "masked")
    nc = bass.Bass()
    qT = nc.dram_tensor("qT", [D, T], F32R, kind="ExternalInput")
    kTt = nc.dram_tensor("kT", [D, T], F32R, kind="ExternalInput")
    vT = nc.dram_tensor("vT", [D, T], F32R, kind="ExternalInput")
    wq = nc.dram_tensor("wq", [D, G], F32R, kind="ExternalInput")
    wk = nc.dram_tensor("wk", [D, G], F32R, kind="ExternalInput")
    wv = nc.dram_tensor("wv", [D, G], F32R, kind="ExternalInput")
    wo = nc.dram_tensor("wo", [G, D], F32R, kind="ExternalInput")
    out = nc.dram_tensor("out", [T, D], F32, kind="ExternalOutput")
    rdram = nc.dram_tensor("rdram", [HG, T], F32)
    maskd = None
    if mode == "masked":
        maskd = nc.dram_tensor("maskT", [T, T], F32, kind="ExternalInput")

    with tile.TileContext(nc) as tc:
        with (
            tc.tile_pool(name="sing", bufs=1) as sing,
            tc.tile_pool(name="etp", bufs=2) as etp,
            tc.tile_pool(name="ost", bufs=2) as ostp,
        ):
            # ---------------- static/constant tiles ----------------
            wq_s = sing.tile([128, NKT * G], F32R)
            wk_s = sing.tile([128, NKT * G], F32R)
            wv_s = sing.tile([128, NKT * G], F32R)
            for w_s, w_d in ((wq_s, wq), (wk_s, wk), (wv_s, wv)):
                nc.gpsimd.dma_start(
                    out=w_s[:].rearrange("p (kk m) -> p kk m", kk=NKT),
                    in_=w_d[:].rearrange("(kk p) m -> p kk m", p=128))
            wo_s = sing.tile([128, 2 * D], F32R)
            for p in range(2):
                nc.gpsimd.dma_start(out=wo_s[:, p * D:(p + 1) * D],
                                    in_=wo[p * 128:(p + 1) * 128, :])
            ident_f = sing.tile([128, 128], F32)
            make_identity(nc, ident_f[:])
            ident = sing.tile([128, 128], F32R)
            nc.vector.tensor_copy(ident[:], ident_f[:])
            ones_sb = sing.tile([128, NT], F32)
            nc.vector.memset(ones_sb[:], 1.0)
            if mode == "causal":
                masks = sing.tile([128, 4 * TQ], F32)
                nc.gpsimd.memset(masks[:], 0.0)
                for i in range(4):
                    nc.gpsimd.affine_select(
                        out=masks[:, i * TQ:(i + 1) * TQ],
                        in_=masks[:, i * TQ:(i + 1) * TQ],
                        compare_op=mybir.AluOpType.is_ge,
                        fill=NEG, base=-(128 * i), channel_multiplier=-1,
                        pattern=[[1, TQ]])

            qt = [sing.tile([128, T], F32R, name=f"qt{p}") for p in range(2)]
            kt = [sing.tile([128, T], F32R, name=f"kt{p}") for p in range(2)]
            va = [sing.tile([128, NT * (DK + 1)], F32R, name=f"va{h}")
                  for h in range(HG)]
            # aug-last for all heads: rowsum lands on psum partition 64; the
            # two heads of a pair use different column ranges of rs.
            for h in range(HG):
                nc.vector.tensor_copy(va[h][:, DK::DK + 1], ones_sb[:])

            # ---------------- phase 1: projections ----------------
            ph1_cm = tc.tile_pool(name="ph1", bufs=1)
            xin_cm = tc.tile_pool(name="xin", bufs=3)
            ph1 = ph1_cm.__enter__(); xin = xin_cm.__enter__()
            vt = [ph1.tile([128, T], F32R, name=f"vt{p}") for p in range(2)]
            with nc.named_scope("proj"), \
                 tc.tile_pool(name="pps", bufs=1, space="PSUM") as pps:
                for name, w_s, src, dst in (("q", wq_s, qT, qt),
                                            ("k", wk_s, kTt, kt),
                                            ("v", wv_s, vT, vt)):
                    psum = [pps.tile([128, TQ], F32, name=f"pp{name}{i}",
                                     tag=f"pp{i}")
                            for i in range(8)]
                    for kk in range(NKT):
                        xk = xin.tile([128, T], F32R, name=f"x{name}{kk}",
                                      tag="xin")
                        nc.gpsimd.dma_start(
                            out=xk, in_=src[kk * 128:(kk + 1) * 128, :])
                        for m in range(2):
                            for n in range(NS):
                                nc.tensor.matmul(
                                    psum[m * NS + n][:],
                                    w_s[:, kk * G + m * 128: kk * G + (m + 1) * 128],
                                    xk[:, n * TQ:(n + 1) * TQ],
                                    start=(kk == 0), stop=(kk == NKT - 1))
                    for m in range(2):
                        for n in range(NS):
                            nc.vector.tensor_copy(
                                dst[m][:, n * TQ:(n + 1) * TQ],
                                psum[m * NS + n][:])

            # V_aug: PE-transpose vt [dv 64, tk 128] -> [tk 128, dv 64]
            with nc.named_scope("vaug"), \
                 tc.tile_pool(name="tps", bufs=4, space="PSUM") as tps:
                for h in range(HG):
                    p, half = h // 2, h % 2
                    off = 0
                    for t in range(NT):
                        tp = tps.tile([128, DK], F32R, name=f"tp{h}_{t}", tag="tp")
                        nc.tensor.transpose(
                            tp[:], vt[p][half * DK:(half + 1) * DK,
                                         t * 128:(t + 1) * 128],
                            ident[half * DK:(half + 1) * DK,
                                  half * DK:(half + 1) * DK])
                        nc.vector.tensor_copy(
                            va[h][:, t * (DK + 1) + off: t * (DK + 1) + off + DK],
                            tp[:])
            xin_cm.__exit__(None, None, None)
            ph1_cm.__exit__(None, None, None)

            # ---------------- phase 2: attention ----------------
            otrs_cm = tc.tile_pool(name="otrs", bufs=1)
            otrs = otrs_cm.__enter__()
            otu = [otrs.tile([128, T], F32R, name=f"otu{p}") for p in range(2)]
            rs = [otrs.tile([128, 2 * T], F32, name=f"rs{p}") for p in range(2)]
            with nc.named_scope("attn"), \
                 tc.tile_pool(name="sps", bufs=2, space="PSUM") as sps, \
                 tc.tile_pool(name="ops", bufs=2, space="PSUM") as ops, \
                 tc.tile_pool(name="mtp", bufs=4) as mtp:
                for h in range(HG):
                    p, half = h // 2, h % 2
                    po = half * DK
                    for s in range(NS):
                        na = _n_alive(s, mode)
                        ot_ps = ops.tile([DK + 1, TQ], F32, name=f"ot{h}_{s}",
                                         tag="ot")
                        for tp2 in range(na // 2):
                            s_ps = sps.tile([128, 2 * TQ], F32,
                                            name=f"s{h}_{s}_{tp2}", tag="s")
                            for j in range(2):
                                t = 2 * tp2 + j
                                nc.tensor.matmul(
                                    s_ps[:, j * TQ:(j + 1) * TQ],
                                    kt[p][po:po + DK, t * 128:(t + 1) * 128],
                                    qt[p][po:po + DK, s * TQ:(s + 1) * TQ],
                                    start=True, stop=True)
                                if mode == "causal" and t >= (TQ // 128) * s:
                                    d = (128 * t - TQ * s) // 128
                                    nc.vector.tensor_add(
                                        s_ps[:, j * TQ:(j + 1) * TQ],
                                        s_ps[:, j * TQ:(j + 1) * TQ],
                                        masks[:, d * TQ:(d + 1) * TQ])
                                elif mode == "masked":
                                    mt = mtp.tile([128, TQ], F32,
                                                  name=f"mt{h}{s}{t}", tag="mt")
                                    nc.gpsimd.dma_start(
                                        out=mt,
                                        in_=maskd[t * 128:(t + 1) * 128,
                                                  s * TQ:(s + 1) * TQ])
                                    nc.vector.tensor_add(
                                        s_ps[:, j * TQ:(j + 1) * TQ],
                                        s_ps[:, j * TQ:(j + 1) * TQ], mt[:])
                            et = etp.tile([128, 2 * TQ], F32R,
                                          name=f"et{h}_{s}_{tp2}", tag="et")
                            nc.scalar.activation(
                                et[:], s_ps[:],
                                mybir.ActivationFunctionType.Exp,
                                scale=1.0 / np.sqrt(DK))
                            for j in range(2):
                                t = 2 * tp2 + j
                                nc.tensor.matmul(
                                    ot_ps[:],
                                    va[h][:, t * (DK + 1):(t + 1) * (DK + 1)],
                                    et[:, j * TQ:(j + 1) * TQ],
                                    start=(t == 0), stop=(t == na - 1))
                        nc.vector.tensor_copy(
                            otu[p][po:po + DK, s * TQ:(s + 1) * TQ],
                            ot_ps[0:DK, :])
                        nc.vector.tensor_copy(
                            rs[p][DK:DK + 1, half * T + s * TQ:
                                  half * T + (s + 1) * TQ],
                            ot_ps[DK:DK + 1, :])

            # ---------------- normalization ----------------
            with nc.named_scope("norm"):
                for p in range(2):
                    nc.gpsimd.dma_start(out=rdram[2 * p:2 * p + 1, :],
                                        in_=rs[p][DK:DK + 1, 0:T])
                    nc.gpsimd.dma_start(out=rdram[2 * p + 1:2 * p + 2, :],
                                        in_=rs[p][DK:DK + 1, T:2 * T])
                with tc.tile_pool(name="rbcp", bufs=1) as rbcp:
                    for p in range(2):
                        rbc = rbcp.tile([128, T], F32, name=f"rbc{p}",
                                        tag="rbc")
                        nc.gpsimd.dma_start(
                            out=rbc[0:DK, :],
                            in_=rdram[2 * p:2 * p + 1, :].to_broadcast((DK, T)))
                        nc.gpsimd.dma_start(
                            out=rbc[DK:128, :],
                            in_=rdram[2 * p + 1:2 * p + 2, :].to_broadcast((DK, T)))
                        nc.vector.reciprocal(rbc[:], rbc[:])
                        nc.vector.tensor_mul(otu[p][:], otu[p][:], rbc[:])

            # ---------------- phase 3: output projection ----------------
            with nc.named_scope("outproj"), \
                 tc.tile_pool(name="fps", bufs=2, space="PSUM") as fps:
                for m in range(NT):
                    o_sb = ostp.tile([128, D], F32, name=f"os{m}", tag="os")
                    for n in range(2):
                        o_ps = fps.tile([128, TQ], F32, name=f"op{m}_{n}",
                                        tag="op")
                        for p in range(2):
                            nc.tensor.matmul(
                                o_ps[:],
                                otu[p][:, m * 128:(m + 1) * 128],
                                wo_s[:, p * D + n * TQ: p * D + (n + 1) * TQ],
                                start=(p == 0), stop=(p == 1))
                        nc.vector.tensor_copy(o_sb[:, n * TQ:(n + 1) * TQ],
                                              o_ps[:])
                    nc.gpsimd.dma_start(out=out[m * 128:(m + 1) * 128, :],
                                        in_=o_sb[:])
            otrs_cm.__exit__(None, None, None)

    split_multi_waits(nc)
    return nc


def _detect_mode(mask):
    if mask.all():
        return "dense"
    if np.array_equal(mask, np.tril(np.ones((T, T), dtype=bool))):
        return "causal"
    return "masked"


def kernel(q, k, v, mask, Wq, Wk, Wv, Wo, _trace=False, _trace_kwargs=None):
    mode = _detect_mode(np.asarray(mask))
    nc = build_program(mode)

    in_maps = []
    for c in range(8):
        b, g = c // 4, c % 4
        im = {
            "qT": np.ascontiguousarray(q[b].T.astype(np.float32)),
            "kT": np.ascontiguousarray(k[b].T.astype(np.float32)),
            "vT": np.ascontiguousarray(v[b].T.astype(np.float32)),
            "wq": np.ascontiguousarray(Wq[:, g * G:(g + 1) * G].astype(np.float32)),
            "wk": np.ascontiguousarray(Wk[:, g * G:(g + 1) * G].astype(np.float32)),
            "wv": np.ascontiguousarray(Wv[:, g * G:(g + 1) * G].astype(np.float32)),
            "wo": np.ascontiguousarray(Wo[g * G:(g + 1) * G, :].astype(np.float32)),
        }
        if mode == "masked":
            im["maskT"] = np.ascontiguousarray(
                np.where(mask, 0.0, NEG).astype(np.float32).T)
        in_maps.append(im)

    res = run_bass_kernel_spmd(nc, in_maps, list(range(8)), trace=_trace,
                               **(_trace_kwargs or {}))
    outs = [res.results[c]["out"] for c in range(8)]
    full = np.stack([outs[4 * b] + outs[4 * b + 1] + outs[4 * b + 2]
                     + outs[4 * b + 3] for b in range(B)])
    if _trace:
        return full.astype(np.float32), res
    return full.astype(np.float32)


# revision 52
# speedup vs baseline: 407.9552x; 407.9552x over previous
"""Multi-head causal attention (B=2, T=2048, D=1024, H=16, dk=dv=64) on 8 NeuronCores.

Sharding: data parallel over batch (2) x tensor parallel over heads (4 groups of 4).
Core c handles batch c//4, heads [4*(c%4), 4*(c%4)+4). Each core computes the
partial output sum over its 4 heads; host adds the 4 partials per batch.

Per-core pipeline (everything transposed so no activation transposes are needed):
  QT/KT [256, T] = W.T @ xT        (fp32r matmuls, PSUM k-accumulation)
  VT    [256, T] -> PE-transpose -> V_aug [T, 65] per head (65th col = ones)
  per head, per tq-slice (512), per tk-tile (128), causal-skipped:
    ST block [tk 128, tq 512] = KT_h^T-slice.T @ QT_h      (K=64)
    diag blocks: += additive -1e30 mask (DVE on PSUM)
    ET = exp(0.125 * ST)  (ACT, PSUM->SBUF fp32r, batched over 2 blocks)
    OT_aug [65, 512] += V_aug_h.T-slice @ ET   (fused rowsum via ones col)
  normalization: rowsums -> DRAM bounce broadcast -> reciprocal -> OT scale
  out [T, 1024] = OT.T @ Wo  (partial over this core's 4 heads)
"""
import sys

sys.path.insert(0, "/opt/trn_rl_repo")

import functools
import os
import ml_dtypes
import numpy as np

import concourse.bass as bass
import concourse.tile as tile
from concourse import mybir
from concourse.bass_utils import run_bass_kernel_spmd

B, T, D = 2, 2048, 1024
H, DK = 16, 64            # total heads
HG = 4                    # heads per core
G = HG * DK               # 256: per-core column group width
NKT = D // 128            # 8 k-tiles of the model dim
NT = T // 128             # 16 tk tiles
NS = 4                    # tq slices
TQ = T // NS              # 512
NEG = -1e30
F32 = mybir.dt.float32
F32R = mybir.dt.float32r
BF16 = mybir.dt.bfloat16
IN_DT = BF16  # dtype for x / Wq / Wk / Wv (projection operands)


def split_multi_waits(nc, max_waits=1):
    """This walrus build has tiny per-instruction sync-wait slot limits (1 for
    fp32r matmul LW, ~2 for CTRL). Move excess waits onto preceding same-engine
    NOPs - identical semantics since each engine executes serially."""
    for func in nc.m.functions:
        for bb in func.blocks:
            out = []
            for inst in list(bb.instructions):
                si = inst.sync_info
                waits = list(si.on_wait) if (si and si.on_wait) else []
                if len(waits) > max_waits:
                    extra, keep = waits[:-max_waits], waits[-max_waits:]
                    for j, w in enumerate(extra):
                        nop = mybir.InstNoOp(name=f"{inst.name}-ws{j}")
                        nop.engine = inst.engine
                        nop.sync_info = mybir.SyncInfo(on_wait=[w], on_update=[])
                        out.append(nop)
                    inst.sync_info = mybir.SyncInfo(
                        on_wait=keep, on_update=list(si.on_update or []))
                out.append(inst)
            bb.instructions = out


def _n_alive(s, mode):
    """Number of tk tiles needed for tq slice s."""
    return NT if mode != "causal" else (TQ // 128) * (s + 1)


@functools.lru_cache(maxsize=4)
def build_program(mode, _env=None):
    assert mode in ("causal", "dense", "masked")
    nc = bass.Bass()
    qT = nc.dram_tensor("qT", [D, T], IN_DT, kind="ExternalInput")
    kTt = nc.dram_tensor("kT", [D, T], IN_DT, kind="ExternalInput")
    vT = nc.dram_tensor("vT", [D, T], IN_DT, kind="ExternalInput")
    # weights pre-packed on host into SBUF layout: [128, NKT*G] with
    # partition p holding wq[kk*128+p, :] at cols [kk*G, (kk+1)*G)
    wq = nc.dram_tensor("wq", [128, NKT * G], IN_DT, kind="ExternalInput")
    wk = nc.dram_tensor("wk", [128, NKT * G], IN_DT, kind="ExternalInput")
    wv = nc.dram_tensor("wv", [128, NKT * G], IN_DT, kind="ExternalInput")
    wo = nc.dram_tensor("wo", [128, 2 * D], BF16, kind="ExternalInput")
    out = nc.dram_tensor("out", [T, D], F32, kind="ExternalOutput")
    rdram = nc.dram_tensor("rdram", [HG, T], F32)
    DBG = bool(int(os.environ.get("KDBG", "0")))
    dbg = {}
    if DBG:
        for nm, shape, dt_ in [("dqt0", [128, T], BF16), ("dkt0", [128, T], BF16),
                               ("dva0", [128, NT * (DK + 1)], BF16),
                               ("dotu0", [128, T], F32R),
                               ("drd", [HG, T], F32),
                               ("dmasks", [128, 4 * TQ], F32),
                               ("det", [128, 2 * TQ], BF16),
                               ("det1", [128, 2 * TQ], BF16),
                               ("dva1", [128, NT * (DK + 1)], BF16),
                               ("dsps", [128, 2 * TQ], F32),
                               ("dsps1", [128, 2 * TQ], F32)]:
            dbg[nm] = nc.dram_tensor(nm, shape, dt_, kind="ExternalOutput")
    maskd = None
    if mode == "masked":
        maskd = nc.dram_tensor("maskT", [T, T], BF16, kind="ExternalInput")

    with tile.TileContext(nc) as tc:
        with (
            tc.tile_pool(name="sing", bufs=1) as sing,
            tc.tile_pool(name="xbig", bufs=1) as xbig,
            tc.tile_pool(name="etp", bufs=10) as etp,
            tc.tile_pool(name="ost", bufs=4) as ostp,
        ):
            # ---------------- constants ----------------
            wq_s = sing.tile([128, NKT * G], IN_DT)
            wk_s = sing.tile([128, NKT * G], IN_DT)
            wv_s = sing.tile([128, NKT * G], IN_DT)
            for w_s, w_d in ((wq_s, wq), (wk_s, wk), (wv_s, wv)):
                nc.sync.dma_start(out=w_s[:], in_=w_d[:])
            wo_s = sing.tile([128, 2 * D], BF16)
            nc.sync.dma_start(out=wo_s[:], in_=wo[:])
            ones_sb = sing.tile([128, NT], BF16)
            nc.vector.memset(ones_sb[:], 1.0)
            if mode == "causal":
                # multiplicative 0/1 masks (bf16), applied to ET post-exp
                masks = sing.tile([128, 4 * TQ], BF16)
                nc.gpsimd.memset(masks[:], 1.0)
                for i in range(4):
                    nc.gpsimd.affine_select(
                        out=masks[:, i * TQ:(i + 1) * TQ],
                        in_=masks[:, i * TQ:(i + 1) * TQ],
                        compare_op=mybir.AluOpType.is_ge,
                        fill=0.0, base=-(128 * i), channel_multiplier=-1,
                        pattern=[[1, TQ]])

            qt = [sing.tile([128, T], BF16, name=f"qt{p}") for p in range(2)]
            kt = [sing.tile([128, T], BF16, name=f"kt{p}") for p in range(2)]
            va = [sing.tile([128, NT * (DK + 1)], BF16, name=f"va{h}")
                  for h in range(HG)]
            # aug-last: ones col at 64 of each 65-wide group (rowsum row)
            for h in range(HG):
                nc.vector.tensor_copy(va[h][:, DK::DK + 1], ones_sb[:])

            # ------- fused projections + attention (single scheduling region)
            otrs_cm = tc.tile_pool(name="otrs", bufs=1)
            otrs = otrs_cm.__enter__()
            otu = [[otrs.tile([128, TQ], BF16, name=f"otu{p}_{s}")
                    for s in range(NS)] for p in range(2)]
            with nc.named_scope("attn"), \
                 tc.tile_pool(name="sps", bufs=2, space="PSUM") as sps, \
                 tc.tile_pool(name="ops", bufs=2, space="PSUM") as ops, \
                 tc.tile_pool(name="mtp", bufs=4) as mtp, \
                 tc.tile_pool(name="rrp", bufs=4) as rrp:
                def emit_proj(xr, w_s, dst, m, only_n=None):
                    for n in ([only_n] if only_n is not None else range(NS)):
                        ps = qkps.tile([128, TQ], F32,
                                       name=f"pj{xr[0].tensor.name}{m}{n}",
                                       tag="qk")
                        for kk in range(NKT):
                            nc.tensor.matmul(
                                ps[:],
                                w_s[:, kk * G + m * 128: kk * G + (m + 1) * 128],
                                xr[kk][:, n * TQ:(n + 1) * TQ],
                                start=(kk == 0), stop=(kk == NKT - 1))
                        nc.vector.tensor_copy(
                            dst[m][:, n * TQ:(n + 1) * TQ], ps[:])

                def emit_head(h, s_list=None, ot_pools=None):
                    p, half = h // 2, h % 2
                    po = half * DK
                    for s in (range(NS) if s_list is None else s_list):
                        na = _n_alive(s, mode)
                        pool_i = (ot_pools or [ops])
                        opool = pool_i[s % len(pool_i)]
                        ot_ps = opool.tile([DK + 1, TQ], F32, name=f"ot{h}_{s}",
                                           tag=f"ot{opool.name}")
                        for tp2 in range(na // 2):
                            s_ps = sps.tile([128, 2 * TQ], F32,
                                            name=f"s{h}_{s}_{tp2}", tag="s")
                            # diag blocks: cols f < 128*d are masked for every
                            # partition -> skip in S/exp/mask/O. t=0 is always
                            # full width, so PSUM accumulation start covers all.
                            c0s, ds = [], []
                            for j in range(2):
                                t = 2 * tp2 + j
                                if mode == "causal" and t >= (TQ // 128) * s:
                                    d = (128 * t - TQ * s) // 128
                                    ds.append(d); c0s.append(128 * d)
                                else:
                                    ds.append(None); c0s.append(0)
                            for j in range(2):
                                t = 2 * tp2 + j
                                c0 = c0s[j]
                                nc.tensor.matmul(
                                    s_ps[:, j * TQ + c0:(j + 1) * TQ],
                                    kt[p][po:po + DK, t * 128:(t + 1) * 128],
                                    qt[p][po:po + DK, s * TQ + c0:(s + 1) * TQ],
                                    start=True, stop=True)
                            et = etp.tile([128, 2 * TQ], BF16,
                                          name=f"et{h}_{s}_{tp2}", tag="et")
                            if DBG and h == 0 and s == 0 and tp2 == 0:
                                dsp = sing.tile([128, 2 * TQ], F32)
                                nc.vector.tensor_copy(dsp[:], s_ps[:])
                                nc.sync.dma_start(out=dbg["dsps"][:], in_=dsp[:])
                            if DBG and h == 1 and s == 0 and tp2 == 0:
                                dsp1 = sing.tile([128, 2 * TQ], F32)
                                nc.vector.tensor_copy(dsp1[:], s_ps[:])
                                nc.sync.dma_start(out=dbg["dsps1"][:], in_=dsp1[:])
                            # One exp instruction costs ~352 extra cycles;
                            # splitting to skip dead columns only pays off when
                            # the skip is > 128 cols. For small c0 exp the dead
                            # region too (harmless: the O-matmul never reads
                            # it), starting at min(c0s).
                            if max(c0s) <= 128:
                                cm = min(c0s)
                                nc.scalar.activation(
                                    et[:, cm:2 * TQ], s_ps[:, cm:2 * TQ],
                                    mybir.ActivationFunctionType.Exp,
                                    scale=1.0 / np.sqrt(DK))
                            else:
                                for j in range(2):
                                    c0 = c0s[j]
                                    nc.scalar.activation(
                                        et[:, j * TQ + c0:(j + 1) * TQ],
                                        s_ps[:, j * TQ + c0:(j + 1) * TQ],
                                        mybir.ActivationFunctionType.Exp,
                                        scale=1.0 / np.sqrt(DK))
                            for j in range(2):
                                t = 2 * tp2 + j
                                if ds[j] is not None:
                                    d, c0 = ds[j], c0s[j]
                                    nc.vector.tensor_mul(
                                        et[:, j * TQ + c0:j * TQ + c0 + 128],
                                        et[:, j * TQ + c0:j * TQ + c0 + 128],
                                        masks[:, d * TQ + c0:d * TQ + c0 + 128])
                                elif mode == "masked":
                                    mt = mtp.tile([128, TQ], BF16,
                                                  name=f"mt{h}{s}{t}", tag="mt")
                                    nc.sync.dma_start(
                                        out=mt,
                                        in_=maskd[t * 128:(t + 1) * 128,
                                                  s * TQ:(s + 1) * TQ])
                                    nc.vector.tensor_mul(
                                        et[:, j * TQ:(j + 1) * TQ],
                                        et[:, j * TQ:(j + 1) * TQ], mt[:])
                            if DBG and h == 0 and s == 0 and tp2 == 0:
                                nc.sync.dma_start(out=dbg["det"][:], in_=et[:])
                            if DBG and h == 1 and s == 0 and tp2 == 0:
                                nc.sync.dma_start(out=dbg["det1"][:], in_=et[:])
                            for j in range(2):
                                t = 2 * tp2 + j
                                c0 = c0s[j]
                                nc.tensor.matmul(
                                    ot_ps[:, c0:TQ],
                                    va[h][:, t * (DK + 1):(t + 1) * (DK + 1)],
                                    et[:, j * TQ + c0:(j + 1) * TQ],
                                    start=(t == 0), stop=(t == na - 1))
                        nc.vector.tensor_copy(
                            otu[p][s][po:po + DK, :], ot_ps[0:DK, :])
                        rr = rrp.tile([128, TQ], F32, name=f"rr{h}_{s}",
                                      tag="rr")
                        nc.vector.tensor_copy(rr[DK:DK + 1, :],
                                              ot_ps[DK:DK + 1, :])
                        nc.sync.dma_start(
                            out=rdram[h:h + 1, s * TQ:(s + 1) * TQ],
                            in_=rr[DK:DK + 1, :])
                        if half == 1:
                            # both heads of the pair done for this slice:
                            # broadcast both rowsums, one recip + one mul.
                            rb = rrp.tile([128, TQ], F32, name=f"rb{h}_{s}",
                                          tag="rb")
                            nc.gpsimd.dma_start(
                                out=rb[0:DK, :],
                                in_=rdram[h - 1:h, s * TQ:(s + 1) * TQ]
                                .to_broadcast((DK, TQ)))
                            nc.gpsimd.dma_start(
                                out=rb[DK:128, :],
                                in_=rdram[h:h + 1, s * TQ:(s + 1) * TQ]
                                .to_broadcast((DK, TQ)))
                            nc.vector.reciprocal(rb[:], rb[:])
                            nc.vector.tensor_mul(
                                otu[p][s][:], otu[p][s][:], rb[:])

                # V first (heads' O-matmuls read va; emission order defines
                # RAW deps), then pair-0 projections + head 0, etc.
                # per-k-tile input tiles; DMA issue order v,q,k matches the
                # PE stream order (V-matmuls head the stream as the scheduler
                # preserves emission order per engine).
                vTr = [xbig.tile([128, T], IN_DT, name=f"vTr{kk}",
                                 tag=f"xv{kk}") for kk in range(NKT)]
                for kk in range(NKT):
                    nc.sync.dma_start(out=vTr[kk],
                                      in_=vT[kk * 128:(kk + 1) * 128, :])
                qTr = [xbig.tile([128, T], IN_DT, name=f"qTr{kk}", tag=f"xq{kk}")
                       for kk in range(NKT)]
                for kk in range(NKT):
                    nc.sync.dma_start(out=qTr[kk],
                                      in_=qT[kk * 128:(kk + 1) * 128, :])
                kTr = [xbig.tile([128, T], IN_DT, name=f"kTr{kk}", tag=f"xk{kk}")
                       for kk in range(NKT)]
                for kk in range(NKT):
                    nc.sync.dma_start(out=kTr[kk],
                                      in_=kTt[kk * 128:(kk + 1) * 128, :])
                # V in natural layout: per tk-tile, k-inner on resident vTr.
                # Emitted before the heads (O-matmuls read va -> RAW deps need
                # write-before-read emission order) but DEMOTED in priority so
                # the scheduler treats it as PE gap-filler under the ACT-bound
                # attention instead of running it ahead of the S-matmuls.
                with nc.named_scope("vproj"), \
                     tc.high_priority(offset=1000000), \
                     tc.tile_pool(name="vps", bufs=2, space="PSUM") as vps:
                    for t in range(NT):
                        ps = vps.tile([128, G], F32, name=f"v{t}", tag="v")
                        for kk in range(NKT):
                            nc.tensor.matmul(
                                ps[:],
                                vTr[kk][:, t * 128:(t + 1) * 128],
                                wv_s[:, kk * G:(kk + 1) * G],
                                start=(kk == 0), stop=(kk == NKT - 1))
                        for h in range(HG):
                            nc.vector.tensor_copy(
                                va[h][:, t * (DK + 1): t * (DK + 1) + DK],
                                ps[:, h * DK:(h + 1) * DK])
                qkps_cm = tc.tile_pool(name="qkps", bufs=2, space="PSUM")
                qkps = qkps_cm.__enter__()
                # interleave pair-0 projections, head-0 slices, and pair-1
                # projections at slice granularity: head0 slice s needs only
                # the n<=s projection groups, so attention starts as soon as
                # the first groups land; pair-1 projection work fills PE slack
                # under the ACT-bound attention.
                if mode == "causal":
                    # slice s of head 0 only reads the n<=s projection groups
                    for s in range(NS):
                        with nc.named_scope("proj"):
                            emit_proj(qTr, wq_s, qt, 0, only_n=s)
                            emit_proj(kTr, wk_s, kt, 0, only_n=s)
                        emit_head(0, s_list=[s])
                else:
                    # dense/masked: every slice reads all of kt - emit all
                    # pair-0 groups first
                    with nc.named_scope("proj"):
                        emit_proj(qTr, wq_s, qt, 0)
                        emit_proj(kTr, wk_s, kt, 0)
                    emit_head(0)
                # pair-1 projections interleaved into head 1 (pair 0): its
                # longer attention slices hide the projection PE time.
                for s in range(NS):
                    emit_head(1, s_list=[s])
                    with nc.named_scope("proj2"):
                        emit_proj(qTr, wq_s, qt, 1, only_n=s)
                        emit_proj(kTr, wk_s, kt, 1, only_n=s)
                qkps_cm.__exit__(None, None, None)
                with tc.tile_pool(name="ops2", bufs=2, space="PSUM") as ops2:
                    emit_head(2, ot_pools=[ops, ops2])
                # head 3 interleaved with the output projection: outproj for
                # tq slice s needs only otu[*][s], which is final once head 3
                # (the last head) finishes slice s.
                with tc.tile_pool(name="fps", bufs=2, space="PSUM") as fps:
                    for s in range(NS):
                        emit_head(3, s_list=[s])
                        with nc.named_scope("outproj"):
                            for m in range(4 * s, 4 * s + 4):
                                o_sb = ostp.tile([128, D], F32,
                                                 name=f"os{m}", tag="os")
                                for n in range(2):
                                    o_ps = fps.tile([128, TQ], F32,
                                                    name=f"op{m}_{n}", tag="op")
                                    for p in range(2):
                                        nc.tensor.matmul(
                                            o_ps[:],
                                            otu[p][s][:, (m % 4) * 128:
                                                      (m % 4 + 1) * 128],
                                            wo_s[:, p * D + n * TQ:
                                                 p * D + (n + 1) * TQ],
                                            start=(p == 0), stop=(p == 1))
                                    nc.vector.tensor_copy(
                                        o_sb[:, n * TQ:(n + 1) * TQ],
                                        o_ps[:])
                                nc.sync.dma_start(
                                    out=out[m * 128:(m + 1) * 128, :],
                                    in_=o_sb[:])

            if DBG:
                nc.sync.dma_start(out=dbg["dmasks"][:], in_=masks[:])
                nc.sync.dma_start(out=dbg["dqt0"][:], in_=qt[0][:])
                nc.sync.dma_start(out=dbg["dkt0"][:], in_=kt[0][:])
                nc.sync.dma_start(out=dbg["dva0"][:], in_=va[0][:])
                nc.sync.dma_start(out=dbg["dva1"][:], in_=va[1][:])
                nc.sync.dma_start(out=dbg["dotu0"][:], in_=otu[0][:])
                nc.sync.dma_start(out=dbg["drd"][:], in_=rdram[:])

            otrs_cm.__exit__(None, None, None)

    split_multi_waits(nc)
    return nc


def _detect_mode(mask):
    if mask.all():
        return "dense"
    if np.array_equal(mask, np.tril(np.ones((T, T), dtype=bool))):
        return "causal"
    return "masked"


def kernel(q, k, v, mask, Wq, Wk, Wv, Wo, _trace=False, _trace_kwargs=None):
    q, k, v = np.asarray(q), np.asarray(k), np.asarray(v)
    Wq, Wk, Wv, Wo = (np.asarray(Wq), np.asarray(Wk),
                      np.asarray(Wv), np.asarray(Wo))
    mask = np.asarray(mask)
    mode = _detect_mode(mask)
    nc = build_program(mode)

    in_maps = []
    for c in range(8):
        b, g = c // 4, c % 4
        bf = ml_dtypes.bfloat16

        def packw(w):  # [D, G] -> [128, NKT*G] SBUF layout
            return np.ascontiguousarray(
                w.reshape(NKT, 128, G).transpose(1, 0, 2).reshape(128, NKT * G)
                .astype(bf))

        wo_sl = Wo[g * G:(g + 1) * G, :]
        im = {
            "qT": np.ascontiguousarray(q[b].T.astype(bf)),
            "kT": np.ascontiguousarray(k[b].T.astype(bf)),
            "vT": np.ascontiguousarray(v[b].T.astype(bf)),
            "wq": packw(Wq[:, g * G:(g + 1) * G]),
            "wk": packw(Wk[:, g * G:(g + 1) * G]),
            "wv": packw(Wv[:, g * G:(g + 1) * G]),
            "wo": np.ascontiguousarray(
                wo_sl.reshape(2, 128, D).transpose(1, 0, 2).reshape(128, 2 * D)
                .astype(bf)),
        }
        if mode == "masked":
            im["maskT"] = np.ascontiguousarray(
                mask.T.astype(ml_dtypes.bfloat16))
        in_maps.append(im)

    res = run_bass_kernel_spmd(nc, in_maps, list(range(8)), trace=_trace,
                               **(_trace_kwargs or {}))
    outs = [res.results[c]["out"] for c in range(8)]
    full = np.stack([outs[4 * b] + outs[4 * b + 1] + outs[4 * b + 2]
                     + outs[4 * b + 3] for b in range(B)])
    if _trace:
        return full.astype(np.float32), res
    return full.astype(np.float32)


# revision 53
# speedup vs baseline: 417.2532x; 1.0228x over previous
"""Multi-head causal attention (B=2, T=2048, D=1024, H=16, dk=dv=64) on 8 NeuronCores.

Sharding: data parallel over batch (2) x tensor parallel over heads (4 groups of 4).
Core c handles batch c//4, heads [4*(c%4), 4*(c%4)+4). Each core computes the
partial output sum over its 4 heads; host adds the 4 partials per batch.

Per-core pipeline (everything transposed so no activation transposes are needed):
  QT/KT [256, T] = W.T @ xT        (fp32r matmuls, PSUM k-accumulation)
  VT    [256, T] -> PE-transpose -> V_aug [T, 65] per head (65th col = ones)
  per head, per tq-slice (512), per tk-tile (128), causal-skipped:
    ST block [tk 128, tq 512] = KT_h^T-slice.T @ QT_h      (K=64)
    diag blocks: += additive -1e30 mask (DVE on PSUM)
    ET = exp(0.125 * ST)  (ACT, PSUM->SBUF fp32r, batched over 2 blocks)
    OT_aug [65, 512] += V_aug_h.T-slice @ ET   (fused rowsum via ones col)
  normalization: rowsums -> DRAM bounce broadcast -> reciprocal -> OT scale
  out [T, 1024] = OT.T @ Wo  (partial over this core's 4 heads)
"""
import sys

sys.path.insert(0, "/opt/trn_rl_repo")

import functools
import os
import ml_dtypes
import numpy as np

import concourse.bass as bass
import concourse.tile as tile
from concourse import mybir
from concourse.bass_utils import run_bass_kernel_spmd

B, T, D = 2, 2048, 1024
H, DK = 16, 64            # total heads
HG = 4                    # heads per core
G = HG * DK               # 256: per-core column group width
NKT = D // 128            # 8 k-tiles of the model dim
NT = T // 128             # 16 tk tiles
NS = 4                    # tq slices
TQ = T // NS              # 512
NEG = -1e30
F32 = mybir.dt.float32
F32R = mybir.dt.float32r
BF16 = mybir.dt.bfloat16
IN_DT = BF16  # dtype for x / Wq / Wk / Wv (projection operands)


def split_multi_waits(nc, max_waits=1):
    """This walrus build has tiny per-instruction sync-wait slot limits (1 for
    fp32r matmul LW, ~2 for CTRL). Move excess waits onto preceding same-engine
    NOPs - identical semantics since each engine executes serially."""
    for func in nc.m.functions:
        for bb in func.blocks:
            out = []
            for inst in list(bb.instructions):
                si = inst.sync_info
                waits = list(si.on_wait) if (si and si.on_wait) else []
                if len(waits) > max_waits:
                    extra, keep = waits[:-max_waits], waits[-max_waits:]
                    for j, w in enumerate(extra):
                        nop = mybir.InstNoOp(name=f"{inst.name}-ws{j}")
                        nop.engine = inst.engine
                        nop.sync_info = mybir.SyncInfo(on_wait=[w], on_update=[])
                        out.append(nop)
                    inst.sync_info = mybir.SyncInfo(
                        on_wait=keep, on_update=list(si.on_update or []))
                out.append(inst)
            bb.instructions = out


def _n_alive(s, mode):
    """Number of tk tiles needed for tq slice s."""
    return NT if mode != "causal" else (TQ // 128) * (s + 1)


@functools.lru_cache(maxsize=4)
def build_program(mode, _env=None):
    assert mode in ("causal", "dense", "masked")
    nc = bass.Bass()
    qT = nc.dram_tensor("qT", [D, T], IN_DT, kind="ExternalInput")
    kTt = nc.dram_tensor("kT", [D, T], IN_DT, kind="ExternalInput")
    vT = nc.dram_tensor("vT", [D, T], IN_DT, kind="ExternalInput")
    # weights pre-packed on host into SBUF layout: [128, NKT*G] with
    # partition p holding wq[kk*128+p, :] at cols [kk*G, (kk+1)*G)
    wq = nc.dram_tensor("wq", [128, NKT * G], IN_DT, kind="ExternalInput")
    wk = nc.dram_tensor("wk", [128, NKT * G], IN_DT, kind="ExternalInput")
    wv = nc.dram_tensor("wv", [128, NKT * G], IN_DT, kind="ExternalInput")
    wo = nc.dram_tensor("wo", [128, 2 * D], BF16, kind="ExternalInput")
    out = nc.dram_tensor("out", [T, D], F32, kind="ExternalOutput")
    rdram = nc.dram_tensor("rdram", [HG, T], F32)
    DBG = bool(int(os.environ.get("KDBG", "0")))
    dbg = {}
    if DBG:
        for nm, shape, dt_ in [("dqt0", [128, T], BF16), ("dkt0", [128, T], BF16),
                               ("dva0", [128, NT * (DK + 1)], BF16),
                               ("dotu0", [128, T], F32R),
                               ("drd", [HG, T], F32),
                               ("dmasks", [128, 4 * TQ], F32),
                               ("det", [128, 2 * TQ], BF16),
                               ("det1", [128, 2 * TQ], BF16),
                               ("dva1", [128, NT * (DK + 1)], BF16),
                               ("dsps", [128, 2 * TQ], F32),
                               ("dsps1", [128, 2 * TQ], F32)]:
            dbg[nm] = nc.dram_tensor(nm, shape, dt_, kind="ExternalOutput")
    maskd = None
    if mode == "masked":
        maskd = nc.dram_tensor("maskT", [T, T], BF16, kind="ExternalInput")

    with tile.TileContext(nc) as tc:
        with (
            tc.tile_pool(name="sing", bufs=1) as sing,
            tc.tile_pool(name="xbig", bufs=1) as xbig,
            tc.tile_pool(name="etp", bufs=10) as etp,
            tc.tile_pool(name="ost", bufs=4) as ostp,
        ):
            # ---------------- constants ----------------
            wq_s = sing.tile([128, NKT * G], IN_DT)
            wk_s = sing.tile([128, NKT * G], IN_DT)
            wv_s = sing.tile([128, NKT * G], IN_DT)
            # DMA issue order follows need-time: wv before vTr (V-matmuls
            # consume both first); wq/wk after vTr; wo last (outproj only).
            nc.sync.dma_start(out=wv_s[:], in_=wv[:])
            wo_s = sing.tile([128, 2 * D], BF16)
            ones_sb = sing.tile([128, NT], BF16)
            nc.vector.memset(ones_sb[:], 1.0)
            if mode == "causal":
                # multiplicative 0/1 masks (bf16), applied to ET post-exp
                masks = sing.tile([128, 4 * TQ], BF16)
                nc.gpsimd.memset(masks[:], 1.0)
                for i in range(4):
                    nc.gpsimd.affine_select(
                        out=masks[:, i * TQ:(i + 1) * TQ],
                        in_=masks[:, i * TQ:(i + 1) * TQ],
                        compare_op=mybir.AluOpType.is_ge,
                        fill=0.0, base=-(128 * i), channel_multiplier=-1,
                        pattern=[[1, TQ]])

            qt = [sing.tile([128, T], BF16, name=f"qt{p}") for p in range(2)]
            kt = [sing.tile([128, T], BF16, name=f"kt{p}") for p in range(2)]
            va = [sing.tile([128, NT * (DK + 1)], BF16, name=f"va{h}")
                  for h in range(HG)]
            # aug-last: ones col at 64 of each 65-wide group (rowsum row)
            for h in range(HG):
                nc.vector.tensor_copy(va[h][:, DK::DK + 1], ones_sb[:])

            # ------- fused projections + attention (single scheduling region)
            otrs_cm = tc.tile_pool(name="otrs", bufs=1)
            otrs = otrs_cm.__enter__()
            otu = [[otrs.tile([128, TQ], BF16, name=f"otu{p}_{s}")
                    for s in range(NS)] for p in range(2)]
            with nc.named_scope("attn"), \
                 tc.tile_pool(name="sps", bufs=2, space="PSUM") as sps, \
                 tc.tile_pool(name="ops", bufs=2, space="PSUM") as ops, \
                 tc.tile_pool(name="mtp", bufs=4) as mtp, \
                 tc.tile_pool(name="rrp", bufs=4) as rrp:
                def emit_proj(xr, w_s, dst, m, only_n=None):
                    for n in ([only_n] if only_n is not None else range(NS)):
                        ps = qkps.tile([128, TQ], F32,
                                       name=f"pj{xr[0].tensor.name}{m}{n}",
                                       tag="qk")
                        for kk in range(NKT):
                            nc.tensor.matmul(
                                ps[:],
                                w_s[:, kk * G + m * 128: kk * G + (m + 1) * 128],
                                xr[kk][:, n * TQ:(n + 1) * TQ],
                                start=(kk == 0), stop=(kk == NKT - 1))
                        nc.vector.tensor_copy(
                            dst[m][:, n * TQ:(n + 1) * TQ], ps[:])

                def emit_head(h, s_list=None, ot_pools=None):
                    p, half = h // 2, h % 2
                    po = half * DK
                    for s in (range(NS) if s_list is None else s_list):
                        na = _n_alive(s, mode)
                        pool_i = (ot_pools or [ops])
                        opool = pool_i[s % len(pool_i)]
                        ot_ps = opool.tile([DK + 1, TQ], F32, name=f"ot{h}_{s}",
                                           tag=f"ot{opool.name}")
                        for tp2 in range(na // 2):
                            s_ps = sps.tile([128, 2 * TQ], F32,
                                            name=f"s{h}_{s}_{tp2}", tag="s")
                            # diag blocks: cols f < 128*d are masked for every
                            # partition -> skip in S/exp/mask/O. t=0 is always
                            # full width, so PSUM accumulation start covers all.
                            c0s, ds = [], []
                            for j in range(2):
                                t = 2 * tp2 + j
                                if mode == "causal" and t >= (TQ // 128) * s:
                                    d = (128 * t - TQ * s) // 128
                                    ds.append(d); c0s.append(128 * d)
                                else:
                                    ds.append(None); c0s.append(0)
                            for j in range(2):
                                t = 2 * tp2 + j
                                c0 = c0s[j]
                                nc.tensor.matmul(
                                    s_ps[:, j * TQ + c0:(j + 1) * TQ],
                                    kt[p][po:po + DK, t * 128:(t + 1) * 128],
                                    qt[p][po:po + DK, s * TQ + c0:(s + 1) * TQ],
                                    start=True, stop=True)
                            et = etp.tile([128, 2 * TQ], BF16,
                                          name=f"et{h}_{s}_{tp2}", tag="et")
                            if DBG and h == 0 and s == 0 and tp2 == 0:
                                dsp = sing.tile([128, 2 * TQ], F32)
                                nc.vector.tensor_copy(dsp[:], s_ps[:])
                                nc.sync.dma_start(out=dbg["dsps"][:], in_=dsp[:])
                            if DBG and h == 1 and s == 0 and tp2 == 0:
                                dsp1 = sing.tile([128, 2 * TQ], F32)
                                nc.vector.tensor_copy(dsp1[:], s_ps[:])
                                nc.sync.dma_start(out=dbg["dsps1"][:], in_=dsp1[:])
                            # One exp instruction costs ~352 extra cycles;
                            # splitting to skip dead columns only pays off when
                            # the skip is > 128 cols. For small c0 exp the dead
                            # region too (harmless: the O-matmul never reads
                            # it), starting at min(c0s).
                            if max(c0s) <= 128:
                                cm = min(c0s)
                                nc.scalar.activation(
                                    et[:, cm:2 * TQ], s_ps[:, cm:2 * TQ],
                                    mybir.ActivationFunctionType.Exp,
                                    scale=1.0 / np.sqrt(DK))
                            else:
                                for j in range(2):
                                    c0 = c0s[j]
                                    nc.scalar.activation(
                                        et[:, j * TQ + c0:(j + 1) * TQ],
                                        s_ps[:, j * TQ + c0:(j + 1) * TQ],
                                        mybir.ActivationFunctionType.Exp,
                                        scale=1.0 / np.sqrt(DK))
                            for j in range(2):
                                t = 2 * tp2 + j
                                if ds[j] is not None:
                                    d, c0 = ds[j], c0s[j]
                                    nc.vector.tensor_mul(
                                        et[:, j * TQ + c0:j * TQ + c0 + 128],
                                        et[:, j * TQ + c0:j * TQ + c0 + 128],
                                        masks[:, d * TQ + c0:d * TQ + c0 + 128])
                                elif mode == "masked":
                                    mt = mtp.tile([128, TQ], BF16,
                                                  name=f"mt{h}{s}{t}", tag="mt")
                                    nc.sync.dma_start(
                                        out=mt,
                                        in_=maskd[t * 128:(t + 1) * 128,
                                                  s * TQ:(s + 1) * TQ])
                                    nc.vector.tensor_mul(
                                        et[:, j * TQ:(j + 1) * TQ],
                                        et[:, j * TQ:(j + 1) * TQ], mt[:])
                            if DBG and h == 0 and s == 0 and tp2 == 0:
                                nc.sync.dma_start(out=dbg["det"][:], in_=et[:])
                            if DBG and h == 1 and s == 0 and tp2 == 0:
                                nc.sync.dma_start(out=dbg["det1"][:], in_=et[:])
                            for j in range(2):
                                t = 2 * tp2 + j
                                c0 = c0s[j]
                                nc.tensor.matmul(
                                    ot_ps[:, c0:TQ],
                                    va[h][:, t * (DK + 1):(t + 1) * (DK + 1)],
                                    et[:, j * TQ + c0:(j + 1) * TQ],
                                    start=(t == 0), stop=(t == na - 1))
                        nc.vector.tensor_copy(
                            otu[p][s][po:po + DK, :], ot_ps[0:DK, :])
                        rr = rrp.tile([128, TQ], F32, name=f"rr{h}_{s}",
                                      tag="rr")
                        nc.vector.tensor_copy(rr[DK:DK + 1, :],
                                              ot_ps[DK:DK + 1, :])
                        nc.sync.dma_start(
                            out=rdram[h:h + 1, s * TQ:(s + 1) * TQ],
                            in_=rr[DK:DK + 1, :])
                        if half == 1:
                            # both heads of the pair done for this slice:
                            # broadcast both rowsums, one recip + one mul.
                            rb = rrp.tile([128, TQ], F32, name=f"rb{h}_{s}",
                                          tag="rb")
                            nc.gpsimd.dma_start(
                                out=rb[0:DK, :],
                                in_=rdram[h - 1:h, s * TQ:(s + 1) * TQ]
                                .to_broadcast((DK, TQ)))
                            nc.gpsimd.dma_start(
                                out=rb[DK:128, :],
                                in_=rdram[h:h + 1, s * TQ:(s + 1) * TQ]
                                .to_broadcast((DK, TQ)))
                            nc.vector.reciprocal(rb[:], rb[:])
                            nc.vector.tensor_mul(
                                otu[p][s][:], otu[p][s][:], rb[:])

                # V first (heads' O-matmuls read va; emission order defines
                # RAW deps), then pair-0 projections + head 0, etc.
                # per-k-tile input tiles; DMA issue order v,q,k matches the
                # PE stream order (V-matmuls head the stream as the scheduler
                # preserves emission order per engine).
                vTr = [xbig.tile([128, T], IN_DT, name=f"vTr{kk}",
                                 tag=f"xv{kk}") for kk in range(NKT)]
                for kk in range(NKT):
                    nc.sync.dma_start(out=vTr[kk],
                                      in_=vT[kk * 128:(kk + 1) * 128, :])
                nc.sync.dma_start(out=wq_s[:], in_=wq[:])
                nc.sync.dma_start(out=wk_s[:], in_=wk[:])
                qTr = [xbig.tile([128, T], IN_DT, name=f"qTr{kk}", tag=f"xq{kk}")
                       for kk in range(NKT)]
                for kk in range(NKT):
                    nc.sync.dma_start(out=qTr[kk],
                                      in_=qT[kk * 128:(kk + 1) * 128, :])
                kTr = [xbig.tile([128, T], IN_DT, name=f"kTr{kk}", tag=f"xk{kk}")
                       for kk in range(NKT)]
                for kk in range(NKT):
                    nc.sync.dma_start(out=kTr[kk],
                                      in_=kTt[kk * 128:(kk + 1) * 128, :])
                nc.sync.dma_start(out=wo_s[:], in_=wo[:])
                # V in natural layout: per tk-tile, k-inner on resident vTr.
                # Emitted before the heads (O-matmuls read va -> RAW deps need
                # write-before-read emission order) but DEMOTED in priority so
                # the scheduler treats it as PE gap-filler under the ACT-bound
                # attention instead of running it ahead of the S-matmuls.
                with nc.named_scope("vproj"), \
                     tc.high_priority(offset=1000000), \
                     tc.tile_pool(name="vps", bufs=2, space="PSUM") as vps:
                    for t in range(NT):
                        ps = vps.tile([128, G], F32, name=f"v{t}", tag="v")
                        for kk in range(NKT):
                            nc.tensor.matmul(
                                ps[:],
                                vTr[kk][:, t * 128:(t + 1) * 128],
                                wv_s[:, kk * G:(kk + 1) * G],
                                start=(kk == 0), stop=(kk == NKT - 1))
                        for h in range(HG):
                            nc.vector.tensor_copy(
                                va[h][:, t * (DK + 1): t * (DK + 1) + DK],
                                ps[:, h * DK:(h + 1) * DK])
                qkps_cm = tc.tile_pool(name="qkps", bufs=2, space="PSUM")
                qkps = qkps_cm.__enter__()
                # interleave pair-0 projections, head-0 slices, and pair-1
                # projections at slice granularity: head0 slice s needs only
                # the n<=s projection groups, so attention starts as soon as
                # the first groups land; pair-1 projection work fills PE slack
                # under the ACT-bound attention.
                if mode == "causal":
                    # slice s of head 0 only reads the n<=s projection groups
                    for s in range(NS):
                        with nc.named_scope("proj"):
                            emit_proj(qTr, wq_s, qt, 0, only_n=s)
                            emit_proj(kTr, wk_s, kt, 0, only_n=s)
                        emit_head(0, s_list=[s])
                else:
                    # dense/masked: every slice reads all of kt - emit all
                    # pair-0 groups first
                    with nc.named_scope("proj"):
                        emit_proj(qTr, wq_s, qt, 0)
                        emit_proj(kTr, wk_s, kt, 0)
                    emit_head(0)
                # pair-1 projections interleaved into head 1 (pair 0): its
                # longer attention slices hide the projection PE time.
                for s in range(NS):
                    emit_head(1, s_list=[s])
                    with nc.named_scope("proj2"):
                        emit_proj(qTr, wq_s, qt, 1, only_n=s)
                        emit_proj(kTr, wk_s, kt, 1, only_n=s)
                qkps_cm.__exit__(None, None, None)
                with tc.tile_pool(name="ops2", bufs=2, space="PSUM") as ops2:
                    emit_head(2, ot_pools=[ops, ops2])
                # head 3 interleaved with the output projection: outproj for
                # tq slice s needs only otu[*][s], which is final once head 3
                # (the last head) finishes slice s.
                with tc.tile_pool(name="fps", bufs=2, space="PSUM") as fps:
                    for s in range(NS):
                        emit_head(3, s_list=[s])
                        with nc.named_scope("outproj"):
                            for m in range(4 * s, 4 * s + 4):
                                o_sb = ostp.tile([128, D], F32,
                                                 name=f"os{m}", tag="os")
                                for n in range(2):
                                    o_ps = fps.tile([128, TQ], F32,
                                                    name=f"op{m}_{n}", tag="op")
                                    for p in range(2):
                                        nc.tensor.matmul(
                                            o_ps[:],
                                            otu[p][s][:, (m % 4) * 128:
                                                      (m % 4 + 1) * 128],
                                            wo_s[:, p * D + n * TQ:
                                                 p * D + (n + 1) * TQ],
                                            start=(p == 0), stop=(p == 1))
                                    nc.vector.tensor_copy(
                                        o_sb[:, n * TQ:(n + 1) * TQ],
                                        o_ps[:])
                                nc.sync.dma_start(
                                    out=out[m * 128:(m + 1) * 128, :],
                                    in_=o_sb[:])

            if DBG:
                nc.sync.dma_start(out=dbg["dmasks"][:], in_=masks[:])
                nc.sync.dma_start(out=dbg["dqt0"][:], in_=qt[0][:])
                nc.sync.dma_start(out=dbg["dkt0"][:], in_=kt[0][:])
                nc.sync.dma_start(out=dbg["dva0"][:], in_=va[0][:])
                nc.sync.dma_start(out=dbg["dva1"][:], in_=va[1][:])
                nc.sync.dma_start(out=dbg["dotu0"][:], in_=otu[0][:])
                nc.sync.dma_start(out=dbg["drd"][:], in_=rdram[:])

            otrs_cm.__exit__(None, None, None)

    split_multi_waits(nc)
    return nc


def _detect_mode(mask):
    if mask.all():
        return "dense"
    if np.array_equal(mask, np.tril(np.ones((T, T), dtype=bool))):
        return "causal"
    return "masked"


def kernel(q, k, v, mask, Wq, Wk, Wv, Wo, _trace=False, _trace_kwargs=None):
    q, k, v = np.asarray(q), np.asarray(k), np.asarray(v)
    Wq, Wk, Wv, Wo = (np.asarray(Wq), np.asarray(Wk),
                      np.asarray(Wv), np.asarray(Wo))
    mask = np.asarray(mask)
    mode = _detect_mode(mask)
    nc = build_program(mode)

    in_maps = []
    for c in range(8):
        b, g = c // 4, c % 4
        bf = ml_dtypes.bfloat16

        def packw(w):  # [D, G] -> [128, NKT*G] SBUF layout
            return np.ascontiguousarray(
                w.reshape(NKT, 128, G).transpose(1, 0, 2).reshape(128, NKT * G)
                .astype(bf))

        wo_sl = Wo[g * G:(g + 1) * G, :]
        im = {
            "qT": np.ascontiguousarray(q[b].T.astype(bf)),
            "kT": np.ascontiguousarray(k[b].T.astype(bf)),
            "vT": np.ascontiguousarray(v[b].T.astype(bf)),
            "wq": packw(Wq[:, g * G:(g + 1) * G]),
            "wk": packw(Wk[:, g * G:(g + 1) * G]),
            "wv": packw(Wv[:, g * G:(g + 1) * G]),
            "wo": np.ascontiguousarray(
                wo_sl.reshape(2, 128, D).transpose(1, 0, 2).reshape(128, 2 * D)
                .astype(bf)),
        }
        if mode == "masked":
            im["maskT"] = np.ascontiguousarray(
                mask.T.astype(ml_dtypes.bfloat16))
        in_maps.append(im)

    res = run_bass_kernel_spmd(nc, in_maps, list(range(8)), trace=_trace,
                               **(_trace_kwargs or {}))
    outs = [res.results[c]["out"] for c in range(8)]
    full = np.stack([outs[4 * b] + outs[4 * b + 1] + outs[4 * b + 2]
                     + outs[4 * b + 3] for b in range(B)])
    if _trace:
        return full.astype(np.float32), res
    return full.astype(np.float32)
